# revision 13
# baseline (speedup 1.0000x reference)
"""Trainium2 Bass kernel for GQA attention (B=2, S=2048, DIM=2048, H=32, KV=8, HD=64).

Sharding: tensor-parallel over kv heads (TP=4, 2 kv heads / 8 q heads per core)
x data-parallel over batch (DP=2).  Core c = d*4 + t.  Each core computes a
partial out = attn_out_shard @ wo_rows_shard for its batch; the host sums the
4 TP partials per batch.

All host-side work is layout-only: transpose x, permute wq/wk columns into a
RoPE-friendly even/odd layout, cast to bf16, build trig/mask pattern tiles.
"""

import os
import sys

import numpy as np

_REPO = "/opt/trn_rl_repo"
if _REPO not in sys.path:
    sys.path.insert(0, _REPO)

import ml_dtypes  # noqa: E402

BF16 = ml_dtypes.bfloat16

B, S, DIM = 2, 2048, 2048
H, KV, HD = 32, 8, 64
TP, DP = 4, 2
NCORES = TP * DP
HQ = (H // TP) * HD          # 512 q-proj cols per core
HKV = (KV // TP) * HD        # 128 kv-proj cols per core
NKVC = KV // TP              # 2 kv heads per core
NPAIR = (H // TP) // 2       # 4 q-head pairs per core
SQC = 512                    # sq chunk width
NCHUNK = S // SQC
SKT = 128                    # sk tile height
NSKT = S // SKT
GRP = 2                      # sk tiles per score group ([128, 1024] psum)
KT = DIM // 128              # contraction tiles
VW = 130                     # v_sb tile: [0(32) | 1 | 0(31) | v(64) | 1 | pad]

# RoPE layout: within each head's 64 dims -> 64 partitions, quadrant q (32)
# holds pairs 16q..16q+15 as [evens(16) | odds(16)].
_perm = np.empty(64, np.int64)
_freq = np.empty(64, np.int64)
_sgn = np.empty(64, np.float32)
for _p in range(64):
    _q, _j = divmod(_p, 32)
    if _j < 16:
        _i = 16 * _q + _j
        _perm[_p] = 2 * _i
        _sgn[_p] = -1.0
    else:
        _i = 16 * _q + _j - 16
        _perm[_p] = 2 * _i + 1
        _sgn[_p] = 1.0
    _freq[_p] = _i
SHUF = list(range(16, 32)) + list(range(0, 16))

_build_cache = {}
last_exec_time_ns = None
last_trace = None


def _mask_structure(mask):
    """Returns (chunks, patterns): chunks[c] = [(t0, pat_idx|None), ...] over
    groups of GRP sk-tiles; patterns = list of [128, GRP*SQC] float32 0/1."""
    valid = mask[0, 0] == 0.0  # [sq, sk]
    chunks = []
    patterns = []
    pat_keys = {}
    for c in range(NCHUNK):
        glist = []
        for t0 in range(0, NSKT, GRP):
            sub = valid[c * SQC:(c + 1) * SQC, t0 * SKT:(t0 + GRP) * SKT]
            if not sub.any():
                continue
            if sub.all():
                glist.append((t0, None))
                continue
            pat = np.empty((128, GRP * SQC), np.float32)
            for u in range(GRP):
                pat[:, u * SQC:(u + 1) * SQC] = (
                    valid[c * SQC:(c + 1) * SQC,
                          (t0 + u) * SKT:(t0 + u + 1) * SKT].T
                )
            key = pat.tobytes()
            if key not in pat_keys:
                pat_keys[key] = len(patterns)
                patterns.append(pat)
            glist.append((t0, pat_keys[key]))
        chunks.append(glist)
    return chunks, patterns


def _build(chunks, n_pat):
    import concourse.bass as bass  # noqa: F401
    import concourse.mybir as mybir
    from concourse import bacc
    from concourse.tile import TileContext

    F32, BF = mybir.dt.float32, mybir.dt.bfloat16
    MUL = mybir.AluOpType.mult
    ADD = mybir.AluOpType.add
    EXP = mybir.ActivationFunctionType.Exp

    nc = bacc.Bacc()
    xt_e = nc.declare_dram_parameter("xt", [DIM, S], BF, isOutput=False)
    wq_e = nc.declare_dram_parameter("wq", [DIM, HQ], BF, isOutput=False)
    wk_e = nc.declare_dram_parameter("wk", [DIM, HKV], BF, isOutput=False)
    wv_e = nc.declare_dram_parameter("wv", [DIM, HKV], BF, isOutput=False)
    wo_e = nc.declare_dram_parameter("wo", [HQ, DIM], BF, isOutput=False)
    c1_e = nc.declare_dram_parameter("c1", [128, S], BF, isOutput=False)
    c2_e = nc.declare_dram_parameter("c2", [128, S], BF, isOutput=False)
    dm_e = nc.declare_dram_parameter("dmask", [128, n_pat * GRP * SQC], BF,
                                     isOutput=False)
    out_e = nc.declare_dram_parameter("out", [S, DIM], F32, isOutput=True)

    with TileContext(nc) as tc:
        with tc.tile_pool(name="persist", bufs=1) as P:
            q_t = [P.tile([128, S], BF, tag=f"q{j}", name=f"q{j}")
                   for j in range(NPAIR)]
            k_t = P.tile([128, S], BF, tag="kt")
            v_sb = [P.tile([128, NSKT * VW], BF, tag=f"v{g}", name=f"v{g}")
                    for g in range(NKVC)]
            attn = [P.tile([128, S], BF, tag=f"a{j}", name=f"a{j}")
                    for j in range(NPAIR)]
            wo_sb = [P.tile([128, DIM], BF, tag=f"wo{j}", name=f"wo{j}")
                     for j in range(NPAIR)]
            dm_sb = P.tile([128, n_pat * GRP * SQC], BF, tag="dm")

            for j in range(NPAIR):
                nc.sync.dma_start(out=wo_sb[j],
                                  in_=wo_e[128 * j:128 * (j + 1), :])
            nc.sync.dma_start(out=dm_sb, in_=dm_e[:, :])

            # v background: [0(63) | 1 | v | 1 | pad] per sk-tile block
            for g in range(NKVC):
                v3 = v_sb[g].rearrange("p (t w) -> p t w", w=VW)
                nc.vector.memset(v3[:, :, 0:32], 0.0)
                nc.vector.memset(v3[:, :, 32:33], 1.0)
                nc.vector.memset(v3[:, :, 33:64], 0.0)
                nc.vector.memset(v3[:, :, 128:129], 1.0)

            # ---------------- projections ----------------
            with (
                tc.tile_pool(name="xw", bufs=1) as XW,
                tc.tile_pool(name="ropew", bufs=2) as W,
                tc.tile_pool(name="pps", bufs=2, space="PSUM") as PPS,
            ):
                c1_sb = XW.tile([128, S], BF, tag="c1")
                c2_sb = XW.tile([128, S], BF, tag="c2")
                nc.sync.dma_start(out=c1_sb, in_=c1_e[:, :])
                nc.sync.dma_start(out=c2_sb, in_=c2_e[:, :])
                # interleave per-k loads so the first q-proj matmuls can
                # start as soon as (wq[0], x[0]) land
                xt_sb, wq_sb, wk_sb, wv_sb = [], [], [], []
                for k in range(KT):
                    qk_ = XW.tile([128, HQ], BF, tag=f"wq{k}")
                    nc.sync.dma_start(out=qk_,
                                      in_=wq_e[128 * k:128 * (k + 1), :])
                    wq_sb.append(qk_)
                    xk = XW.tile([128, S], BF, tag=f"x{k}")
                    nc.sync.dma_start(out=xk,
                                      in_=xt_e[128 * k:128 * (k + 1), :])
                    xt_sb.append(xk)
                    kk = XW.tile([128, HKV], BF, tag=f"wk{k}")
                    nc.sync.dma_start(out=kk,
                                      in_=wk_e[128 * k:128 * (k + 1), :])
                    wk_sb.append(kk)
                    vk = XW.tile([128, HKV], BF, tag=f"wv{k}")
                    nc.sync.dma_start(out=vk,
                                      in_=wv_e[128 * k:128 * (k + 1), :])
                    wv_sb.append(vk)

                def rope_project(dst, w_tiles, col0):
                    raw = W.tile([128, S], BF, tag="qraw")
                    for c in range(NCHUNK):
                        ps = PPS.tile([128, SQC], F32, tag="pq")
                        for k in range(KT):
                            nc.tensor.matmul(
                                ps,
                                w_tiles[k][:, col0:col0 + 128],
                                xt_sb[k][:, SQC * c:SQC * (c + 1)],
                                start=(k == 0), stop=(k == KT - 1),
                            )
                        nc.vector.tensor_copy(raw[:, SQC * c:SQC * (c + 1)], ps)
                    sh = W.tile([128, S], BF, tag="sh")
                    t1 = W.tile([128, S], BF, tag="t1")
                    nc.vector.stream_shuffle(sh, raw, SHUF)
                    nc.vector.tensor_tensor(t1, raw, c1_sb, MUL)
                    nc.vector.tensor_tensor(sh, sh, c2_sb, MUL)
                    nc.vector.tensor_tensor(dst, t1, sh, ADD)

                for j in range(NPAIR):
                    rope_project(q_t[j], wq_sb, 128 * j)
                rope_project(k_t, wk_sb, 0)

                for t in range(NSKT):
                    psv = PPS.tile([128, HKV], F32, tag="pv")
                    for k in range(KT):
                        nc.tensor.matmul(
                            psv,
                            xt_sb[k][:, SKT * t:SKT * (t + 1)],
                            wv_sb[k],
                            start=(k == 0), stop=(k == KT - 1),
                        )
                    for g in range(NKVC):
                        nc.vector.tensor_copy(
                            v_sb[g][:, VW * t + 64:VW * t + 128],
                            psv[:, 64 * g:64 * (g + 1)],
                        )

            # ---------------- attention + wo ----------------
            with (
                tc.tile_pool(name="attw", bufs=2) as W,
                tc.tile_pool(name="scps", bufs=2, space="PSUM") as SCPS,
                tc.tile_pool(name="avps", bufs=1, space="PSUM") as AVPS,
                tc.tile_pool(name="ops", bufs=2, space="PSUM") as OPS,
            ):
                def wo_tile(s):
                    o_sb = W.tile([128, DIM], F32, tag="osb")
                    for n in range(DIM // 512):
                        pso = OPS.tile([128, 512], F32, tag="pso")
                        for j in range(NPAIR):
                            nc.tensor.matmul(
                                pso,
                                attn[j][:, 128 * s:128 * (s + 1)],
                                wo_sb[j][:, 512 * n:512 * (n + 1)],
                                start=(j == 0), stop=(j == NPAIR - 1),
                            )
                        nc.vector.tensor_copy(o_sb[:, 512 * n:512 * (n + 1)],
                                              pso)
                    nc.gpsimd.dma_start(out=out_e[128 * s:128 * (s + 1), :],
                                        in_=o_sb)

                for c in range(NCHUNK):
                    glist = chunks[c]
                    for j in range(NPAIR):
                        # pair j = (q-head j -> kv 0, q-head j+4 -> kv 1)
                        av_lo = AVPS.tile([128, SQC], F32, tag="avlo")
                        av_hi = AVPS.tile([128, SQC], F32, tag="avhi")
                        for gi, (t0, patk) in enumerate(glist):
                            first = gi == 0
                            last = gi == len(glist) - 1
                            for half in (0, 1):
                                rows = slice(64 * half, 64 * (half + 1))
                                av = av_lo if half == 0 else av_hi
                                sc = SCPS.tile([128, GRP * SQC], F32,
                                               tag="sc", name="sc")
                                for u in range(GRP):
                                    t = t0 + u
                                    nc.tensor.matmul(
                                        sc[:, SQC * u:SQC * (u + 1)],
                                        k_t[rows, SKT * t:SKT * (t + 1)],
                                        q_t[j][rows, SQC * c:SQC * (c + 1)],
                                        start=True, stop=True,
                                    )
                                p = W.tile([128, GRP * SQC], BF,
                                           tag=f"p{half}", name="p")
                                nc.scalar.activation(p, sc, EXP, scale=0.125)
                                if patk is not None:
                                    dslice = dm_sb[:, GRP * SQC * patk:
                                                   GRP * SQC * (patk + 1)]
                                    nc.gpsimd.tensor_tensor(p, p, dslice, MUL)
                                for u in range(GRP):
                                    t = t0 + u
                                    if half == 0:
                                        vv = v_sb[0][:, VW * t + 64:
                                                     VW * t + 129]
                                        o = av[0:65, :]
                                    else:
                                        vv = v_sb[1][:, VW * t:VW * t + 128]
                                        o = av[0:128, :]
                                    nc.tensor.matmul(
                                        o, vv, p[:, SQC * u:SQC * (u + 1)],
                                        start=(first and u == 0),
                                        stop=(last and u == GRP - 1),
                                    )
                        rec_lo = W.tile([1, SQC], F32, tag="reclo")
                        rec_hi = W.tile([1, SQC], F32, tag="rechi")
                        nc.vector.reciprocal(rec_lo[0:1, :], av_lo[64:65, :])
                        nc.vector.reciprocal(rec_hi[0:1, :], av_hi[32:33, :])
                        rb_lo = W.tile([128, SQC], F32, tag="rblo")
                        rb_hi = W.tile([128, SQC], F32, tag="rbhi")
                        nc.gpsimd.partition_broadcast(rb_lo, rec_lo[0:1, :])
                        nc.gpsimd.partition_broadcast(rb_hi, rec_hi[0:1, :])
                        nc.vector.tensor_tensor(
                            attn[j][0:64, SQC * c:SQC * (c + 1)],
                            av_lo[0:64, :], rb_lo[0:64, :], MUL)
                        nc.vector.tensor_tensor(
                            attn[j][64:128, SQC * c:SQC * (c + 1)],
                            av_hi[64:128, :], rb_hi[64:128, :], MUL)
                        if c > 0:
                            wo_tile(4 * (c - 1) + j)
                for j in range(NPAIR):
                    wo_tile(4 * (NCHUNK - 1) + j)

    nc.finalize()
    return nc


def kernel(**inputs):
    global last_exec_time_ns, last_trace
    from concourse.bass_utils import run_bass_kernel_spmd

    x = np.asarray(inputs["x"], np.float32)
    freqs_cos = np.asarray(inputs["freqs_cos"], np.float32)
    freqs_sin = np.asarray(inputs["freqs_sin"], np.float32)
    mask = np.asarray(inputs["mask"], np.float32)
    wq = np.asarray(inputs["wq"], np.float32)
    wk = np.asarray(inputs["wk"], np.float32)
    wv = np.asarray(inputs["wv"], np.float32)
    wo = np.asarray(inputs["wo"], np.float32)

    chunks, patterns = _mask_structure(mask)
    n_pat = max(len(patterns), 1)
    if patterns:
        dmask = np.concatenate(patterns, axis=1).astype(BF16)
    else:
        dmask = np.ones((128, GRP * SQC), np.float32).astype(BF16)

    key = tuple(tuple(g) for g in chunks)
    if key not in _build_cache:
        _build_cache[key] = _build(chunks, n_pat)
    nc = _build_cache[key]

    # trig tiles in pair layout (same for both heads of a pair)
    fi2 = np.tile(_freq, 2)
    sg2 = np.tile(_sgn, 2)
    c1 = freqs_cos.T[fi2].astype(BF16)                      # [128, S]
    c2 = (freqs_sin.T[fi2] * sg2[:, None]).astype(BF16)     # [128, S]

    # pair j holds (q-head j, q-head j+4) so lo half uses kv 0, hi half kv 1
    pair_order = [0, 4, 1, 5, 2, 6, 3, 7]
    q_cols = np.concatenate([64 * pair_order[i] + _perm
                             for i in range(H // TP)])
    o_rows = np.concatenate([np.arange(64 * pair_order[i],
                                       64 * pair_order[i] + 64)
                             for i in range(H // TP)])
    kv_perm = np.concatenate([64 * h + _perm for h in range(KV // TP)])

    in_maps = []
    for d in range(DP):
        xt = np.ascontiguousarray(x[d].T).astype(BF16)
        for t in range(TP):
            wq_s = np.ascontiguousarray(
                wq[:, HQ * t:HQ * (t + 1)][:, q_cols]).astype(BF16)
            wk_s = np.ascontiguousarray(
                wk[:, HKV * t:HKV * (t + 1)][:, kv_perm]).astype(BF16)
            wv_s = np.ascontiguousarray(
                wv[:, HKV * t:HKV * (t + 1)]).astype(BF16)
            wo_s = np.ascontiguousarray(
                wo[HQ * t:HQ * (t + 1), :][o_rows]).astype(BF16)
            in_maps.append({
                "xt": xt, "wq": wq_s, "wk": wk_s, "wv": wv_s, "wo": wo_s,
                "c1": c1, "c2": c2, "dmask": dmask,
            })

    trace = bool(os.environ.get("BASS_KERNEL_TRACE"))
    res = run_bass_kernel_spmd(nc, in_maps, core_ids=list(range(NCORES)),
                               trace=trace)
    last_exec_time_ns = res.exec_time_ns
    last_trace = res
    out = np.empty((B, S, DIM), np.float32)
    for d in range(DP):
        acc = res.results[d * TP]["out"].astype(np.float32)
        for t in range(1, TP):
            acc = acc + res.results[d * TP + t]["out"]
        out[d] = acc
    return out


# revision 14
# speedup vs baseline: 1.3467x; 1.3467x over previous
"""Trainium2 Bass kernel for GQA attention (B=2, S=2048, DIM=2048, H=32, KV=8, HD=64).

Sharding: tensor-parallel over kv heads (TP=4, 2 kv heads / 8 q heads per core)
x data-parallel over batch (DP=2).  Core c = d*4 + t.  Each core computes a
partial out = attn_out_shard @ wo_rows_shard for its batch; the host sums the
4 TP partials per batch.

All host-side work is layout-only: transpose x, permute wq/wk columns into a
RoPE-friendly even/odd layout, cast to bf16, build trig/mask pattern tiles.
"""

import os
import sys

import numpy as np

_REPO = "/opt/trn_rl_repo"
if _REPO not in sys.path:
    sys.path.insert(0, _REPO)

import ml_dtypes  # noqa: E402

BF16 = ml_dtypes.bfloat16

B, S, DIM = 2, 2048, 2048
H, KV, HD = 32, 8, 64
TP, DP = 4, 2
NCORES = TP * DP
HQ = (H // TP) * HD          # 512 q-proj cols per core
HKV = (KV // TP) * HD        # 128 kv-proj cols per core
NKVC = KV // TP              # 2 kv heads per core
NPAIR = (H // TP) // 2       # 4 q-head pairs per core
SQC = 512                    # sq chunk width
NCHUNK = S // SQC
SKT = 128                    # sk tile height
NSKT = S // SKT
GRP = 2                      # sk tiles per score group ([128, 1024] psum)
KT = DIM // 128              # contraction tiles
VW = 130                     # v_sb tile: [0(32) | 1 | 0(31) | v(64) | 1 | pad]

# RoPE layout: within each head's 64 dims -> 64 partitions, quadrant q (32)
# holds pairs 16q..16q+15 as [evens(16) | odds(16)].
_perm = np.empty(64, np.int64)
_freq = np.empty(64, np.int64)
_sgn = np.empty(64, np.float32)
for _p in range(64):
    _q, _j = divmod(_p, 32)
    if _j < 16:
        _i = 16 * _q + _j
        _perm[_p] = 2 * _i
        _sgn[_p] = -1.0
    else:
        _i = 16 * _q + _j - 16
        _perm[_p] = 2 * _i + 1
        _sgn[_p] = 1.0
    _freq[_p] = _i
SHUF = list(range(16, 32)) + list(range(0, 16))

_build_cache = {}
last_exec_time_ns = None
last_trace = None


def _mask_structure(mask):
    """Returns (chunks, patterns): chunks[c] = [(t0, pat_idx|None), ...] over
    groups of GRP sk-tiles; patterns = list of [128, GRP*SQC] float32 0/1."""
    valid = mask[0, 0] == 0.0  # [sq, sk]
    chunks = []
    patterns = []
    pat_keys = {}
    for c in range(NCHUNK):
        glist = []
        for t0 in range(0, NSKT, GRP):
            sub = valid[c * SQC:(c + 1) * SQC, t0 * SKT:(t0 + GRP) * SKT]
            if not sub.any():
                continue
            if sub.all():
                glist.append((t0, None))
                continue
            pat = np.empty((128, GRP * SQC), np.float32)
            for u in range(GRP):
                pat[:, u * SQC:(u + 1) * SQC] = (
                    valid[c * SQC:(c + 1) * SQC,
                          (t0 + u) * SKT:(t0 + u + 1) * SKT].T
                )
            key = pat.tobytes()
            if key not in pat_keys:
                pat_keys[key] = len(patterns)
                patterns.append(pat)
            glist.append((t0, pat_keys[key]))
        chunks.append(glist)
    return chunks, patterns


def _build(chunks, n_pat):
    import concourse.bass as bass  # noqa: F401
    import concourse.mybir as mybir
    from concourse import bacc
    from concourse.tile import TileContext

    F32, BF = mybir.dt.float32, mybir.dt.bfloat16
    MUL = mybir.AluOpType.mult
    ADD = mybir.AluOpType.add
    EXP = mybir.ActivationFunctionType.Exp

    nc = bacc.Bacc()
    xt_e = nc.declare_dram_parameter("xt", [DIM, S], BF, isOutput=False)
    wq_e = nc.declare_dram_parameter("wq", [DIM, HQ], BF, isOutput=False)
    wk_e = nc.declare_dram_parameter("wk", [DIM, HKV], BF, isOutput=False)
    wv_e = nc.declare_dram_parameter("wv", [DIM, HKV], BF, isOutput=False)
    wo_e = nc.declare_dram_parameter("wo", [HQ, DIM], BF, isOutput=False)
    c1_e = nc.declare_dram_parameter("c1", [128, S], BF, isOutput=False)
    c2_e = nc.declare_dram_parameter("c2", [128, S], BF, isOutput=False)
    dm_e = nc.declare_dram_parameter("dmask", [128, n_pat * GRP * SQC], BF,
                                     isOutput=False)
    out_e = nc.declare_dram_parameter("out", [S, DIM], F32, isOutput=True)

    with TileContext(nc) as tc:
        with tc.tile_pool(name="persist", bufs=1) as P:
            q_t = [P.tile([128, S], BF, tag=f"q{j}", name=f"q{j}")
                   for j in range(NPAIR)]
            k_t = P.tile([128, S], BF, tag="kt")
            v_sb = [P.tile([128, NSKT * VW], BF, tag=f"v{g}", name=f"v{g}")
                    for g in range(NKVC)]
            attn = [P.tile([128, S], BF, tag=f"a{j}", name=f"a{j}")
                    for j in range(NPAIR)]
            wo_sb = [P.tile([128, DIM], BF, tag=f"wo{j}", name=f"wo{j}")
                     for j in range(NPAIR)]
            dm_sb = P.tile([128, n_pat * GRP * SQC], BF, tag="dm")

            for j in range(NPAIR):
                nc.sync.dma_start(out=wo_sb[j],
                                  in_=wo_e[128 * j:128 * (j + 1), :])
            nc.sync.dma_start(out=dm_sb, in_=dm_e[:, :])

            # v background: [0(63) | 1 | v | 1 | pad] per sk-tile block
            for g in range(NKVC):
                v3 = v_sb[g].rearrange("p (t w) -> p t w", w=VW)
                nc.vector.memset(v3[:, :, 0:32], 0.0)
                nc.vector.memset(v3[:, :, 32:33], 1.0)
                nc.vector.memset(v3[:, :, 33:64], 0.0)
                nc.vector.memset(v3[:, :, 128:129], 1.0)

            # ---------------- projections ----------------
            with (
                tc.tile_pool(name="xw", bufs=1) as XW,
                tc.tile_pool(name="ropew", bufs=2) as W,
                tc.tile_pool(name="pps", bufs=2, space="PSUM") as PPS,
            ):
                c1_sb = XW.tile([128, S], BF, tag="c1")
                c2_sb = XW.tile([128, S], BF, tag="c2")
                nc.sync.dma_start(out=c1_sb, in_=c1_e[:, :])
                nc.sync.dma_start(out=c2_sb, in_=c2_e[:, :])
                # interleave per-k loads so the first q-proj matmuls can
                # start as soon as (wq[0], x[0]) land
                xt_sb, wq_sb, wk_sb, wv_sb = [], [], [], []
                for k in range(KT):
                    qk_ = XW.tile([128, HQ], BF, tag=f"wq{k}")
                    nc.sync.dma_start(out=qk_,
                                      in_=wq_e[128 * k:128 * (k + 1), :])
                    wq_sb.append(qk_)
                    xk = XW.tile([128, S], BF, tag=f"x{k}")
                    nc.sync.dma_start(out=xk,
                                      in_=xt_e[128 * k:128 * (k + 1), :])
                    xt_sb.append(xk)
                    kk = XW.tile([128, HKV], BF, tag=f"wk{k}")
                    nc.sync.dma_start(out=kk,
                                      in_=wk_e[128 * k:128 * (k + 1), :])
                    wk_sb.append(kk)
                    vk = XW.tile([128, HKV], BF, tag=f"wv{k}")
                    nc.sync.dma_start(out=vk,
                                      in_=wv_e[128 * k:128 * (k + 1), :])
                    wv_sb.append(vk)

                def rope_project(dst, w_tiles, col0):
                    raw = W.tile([128, S], BF, tag="qraw")
                    for c in range(NCHUNK):
                        ps = PPS.tile([128, SQC], F32, tag="pq")
                        for k in range(KT):
                            nc.tensor.matmul(
                                ps,
                                w_tiles[k][:, col0:col0 + 128],
                                xt_sb[k][:, SQC * c:SQC * (c + 1)],
                                start=(k == 0), stop=(k == KT - 1),
                            )
                        nc.vector.tensor_copy(raw[:, SQC * c:SQC * (c + 1)], ps)
                    sh = W.tile([128, S], BF, tag="sh")
                    t1 = W.tile([128, S], BF, tag="t1")
                    nc.vector.stream_shuffle(sh, raw, SHUF)
                    nc.vector.tensor_tensor(t1, raw, c1_sb, MUL)
                    nc.vector.tensor_tensor(sh, sh, c2_sb, MUL)
                    nc.vector.tensor_tensor(dst, t1, sh, ADD)

                for j in range(NPAIR):
                    rope_project(q_t[j], wq_sb, 128 * j)
                rope_project(k_t, wk_sb, 0)

                for t in range(NSKT):
                    psv = PPS.tile([128, HKV], F32, tag="pv")
                    for k in range(KT):
                        nc.tensor.matmul(
                            psv,
                            xt_sb[k][:, SKT * t:SKT * (t + 1)],
                            wv_sb[k],
                            start=(k == 0), stop=(k == KT - 1),
                        )
                    for g in range(NKVC):
                        nc.vector.tensor_copy(
                            v_sb[g][:, VW * t + 64:VW * t + 128],
                            psv[:, 64 * g:64 * (g + 1)],
                        )

            # ---------------- attention + wo ----------------
            with (
                tc.tile_pool(name="attw", bufs=2) as W,
                tc.tile_pool(name="scps", bufs=2, space="PSUM") as SCPS,
                tc.tile_pool(name="avps", bufs=1, space="PSUM") as AVPS,
                tc.tile_pool(name="ops", bufs=2, space="PSUM") as OPS,
            ):
                def wo_tile(s):
                    o_sb = W.tile([128, DIM], F32, tag="osb")
                    for n in range(DIM // 512):
                        pso = OPS.tile([128, 512], F32, tag="pso")
                        for j in range(NPAIR):
                            nc.tensor.matmul(
                                pso,
                                attn[j][:, 128 * s:128 * (s + 1)],
                                wo_sb[j][:, 512 * n:512 * (n + 1)],
                                start=(j == 0), stop=(j == NPAIR - 1),
                            )
                        nc.vector.tensor_copy(o_sb[:, 512 * n:512 * (n + 1)],
                                              pso)
                    nc.gpsimd.dma_start(out=out_e[128 * s:128 * (s + 1), :],
                                        in_=o_sb)

                for c in range(NCHUNK):
                    glist = chunks[c]
                    for j in range(NPAIR):
                        # pair j = (q-head j -> kv 0, q-head j+4 -> kv 1)
                        av_lo = AVPS.tile([128, SQC], F32, tag="avlo")
                        av_hi = AVPS.tile([128, SQC], F32, tag="avhi")
                        for gi, (t0, patk) in enumerate(glist):
                            first = gi == 0
                            last = gi == len(glist) - 1
                            for half in (0, 1):
                                rows = slice(64 * half, 64 * (half + 1))
                                av = av_lo if half == 0 else av_hi
                                sc = SCPS.tile([128, GRP * SQC], F32,
                                               tag="sc", name="sc")
                                for u in range(GRP):
                                    t = t0 + u
                                    nc.tensor.matmul(
                                        sc[:, SQC * u:SQC * (u + 1)],
                                        k_t[rows, SKT * t:SKT * (t + 1)],
                                        q_t[j][rows, SQC * c:SQC * (c + 1)],
                                        start=True, stop=True,
                                    )
                                p = W.tile([128, GRP * SQC], BF,
                                           tag=f"p{half}", name="p")
                                nc.scalar.activation(p, sc, EXP, scale=0.125)
                                if patk is not None:
                                    dslice = dm_sb[:, GRP * SQC * patk:
                                                   GRP * SQC * (patk + 1)]
                                    nc.vector.tensor_tensor(p, p, dslice, MUL)
                                for u in range(GRP):
                                    t = t0 + u
                                    if half == 0:
                                        vv = v_sb[0][:, VW * t + 64:
                                                     VW * t + 129]
                                        o = av[0:65, :]
                                    else:
                                        vv = v_sb[1][:, VW * t:VW * t + 128]
                                        o = av[0:128, :]
                                    nc.tensor.matmul(
                                        o, vv, p[:, SQC * u:SQC * (u + 1)],
                                        start=(first and u == 0),
                                        stop=(last and u == GRP - 1),
                                    )
                        rec_lo = W.tile([1, SQC], F32, tag="reclo")
                        rec_hi = W.tile([1, SQC], F32, tag="rechi")
                        nc.vector.reciprocal(rec_lo[0:1, :], av_lo[64:65, :])
                        nc.vector.reciprocal(rec_hi[0:1, :], av_hi[32:33, :])
                        rb_lo = W.tile([128, SQC], F32, tag="rblo")
                        rb_hi = W.tile([128, SQC], F32, tag="rbhi")
                        nc.gpsimd.partition_broadcast(rb_lo, rec_lo[0:1, :])
                        nc.gpsimd.partition_broadcast(rb_hi, rec_hi[0:1, :])
                        nc.vector.tensor_tensor(
                            attn[j][0:64, SQC * c:SQC * (c + 1)],
                            av_lo[0:64, :], rb_lo[0:64, :], MUL)
                        nc.vector.tensor_tensor(
                            attn[j][64:128, SQC * c:SQC * (c + 1)],
                            av_hi[64:128, :], rb_hi[64:128, :], MUL)
                        if c > 0:
                            wo_tile(4 * (c - 1) + j)
                for j in range(NPAIR):
                    wo_tile(4 * (NCHUNK - 1) + j)

    nc.finalize()
    return nc


def kernel(**inputs):
    global last_exec_time_ns, last_trace
    from concourse.bass_utils import run_bass_kernel_spmd

    x = np.asarray(inputs["x"], np.float32)
    freqs_cos = np.asarray(inputs["freqs_cos"], np.float32)
    freqs_sin = np.asarray(inputs["freqs_sin"], np.float32)
    mask = np.asarray(inputs["mask"], np.float32)
    wq = np.asarray(inputs["wq"], np.float32)
    wk = np.asarray(inputs["wk"], np.float32)
    wv = np.asarray(inputs["wv"], np.float32)
    wo = np.asarray(inputs["wo"], np.float32)

    chunks, patterns = _mask_structure(mask)
    n_pat = max(len(patterns), 1)
    if patterns:
        dmask = np.concatenate(patterns, axis=1).astype(BF16)
    else:
        dmask = np.ones((128, GRP * SQC), np.float32).astype(BF16)

    key = tuple(tuple(g) for g in chunks)
    if key not in _build_cache:
        _build_cache[key] = _build(chunks, n_pat)
    nc = _build_cache[key]

    # trig tiles in pair layout (same for both heads of a pair)
    fi2 = np.tile(_freq, 2)
    sg2 = np.tile(_sgn, 2)
    c1 = freqs_cos.T[fi2].astype(BF16)                      # [128, S]
    c2 = (freqs_sin.T[fi2] * sg2[:, None]).astype(BF16)     # [128, S]

    # pair j holds (q-head j, q-head j+4) so lo half uses kv 0, hi half kv 1
    pair_order = [0, 4, 1, 5, 2, 6, 3, 7]
    q_cols = np.concatenate([64 * pair_order[i] + _perm
                             for i in range(H // TP)])
    o_rows = np.concatenate([np.arange(64 * pair_order[i],
                                       64 * pair_order[i] + 64)
                             for i in range(H // TP)])
    kv_perm = np.concatenate([64 * h + _perm for h in range(KV // TP)])

    in_maps = []
    for d in range(DP):
        xt = np.ascontiguousarray(x[d].T).astype(BF16)
        for t in range(TP):
            wq_s = np.ascontiguousarray(
                wq[:, HQ * t:HQ * (t + 1)][:, q_cols]).astype(BF16)
            wk_s = np.ascontiguousarray(
                wk[:, HKV * t:HKV * (t + 1)][:, kv_perm]).astype(BF16)
            wv_s = np.ascontiguousarray(
                wv[:, HKV * t:HKV * (t + 1)]).astype(BF16)
            wo_s = np.ascontiguousarray(
                wo[HQ * t:HQ * (t + 1), :][o_rows]).astype(BF16)
            in_maps.append({
                "xt": xt, "wq": wq_s, "wk": wk_s, "wv": wv_s, "wo": wo_s,
                "c1": c1, "c2": c2, "dmask": dmask,
            })

    trace = bool(os.environ.get("BASS_KERNEL_TRACE"))
    res = run_bass_kernel_spmd(nc, in_maps, core_ids=list(range(NCORES)),
                               trace=trace)
    last_exec_time_ns = res.exec_time_ns
    last_trace = res
    out = np.empty((B, S, DIM), np.float32)
    for d in range(DP):
        acc = res.results[d * TP]["out"].astype(np.float32)
        for t in range(1, TP):
            acc = acc + res.results[d * TP + t]["out"]
        out[d] = acc
    return out


# revision 18
# speedup vs baseline: 1.4648x; 1.0877x over previous
"""Trainium2 Bass kernel for GQA attention (B=2, S=2048, DIM=2048, H=32, KV=8, HD=64).

Sharding: tensor-parallel over kv heads (TP=4, 2 kv heads / 8 q heads per core)
x data-parallel over batch (DP=2).  Core c = d*4 + t.  Each core computes a
partial out = attn_out_shard @ wo_rows_shard for its batch; the host sums the
4 TP partials per batch.

All host-side work is layout-only: transpose x, permute wq/wk columns into a
RoPE-friendly even/odd layout, cast to bf16, build trig/mask pattern tiles.
"""

import os
import sys

import numpy as np

_REPO = "/opt/trn_rl_repo"
if _REPO not in sys.path:
    sys.path.insert(0, _REPO)

import ml_dtypes  # noqa: E402

BF16 = ml_dtypes.bfloat16

B, S, DIM = 2, 2048, 2048
H, KV, HD = 32, 8, 64
TP, DP = 4, 2
NCORES = TP * DP
HQ = (H // TP) * HD          # 512 q-proj cols per core
HKV = (KV // TP) * HD        # 128 kv-proj cols per core
NKVC = KV // TP              # 2 kv heads per core
NPAIR = (H // TP) // 2       # 4 q-head pairs per core
SQC = 512                    # sq chunk width
NCHUNK = S // SQC
SKT = 128                    # sk tile height
NSKT = S // SKT
GRP = 2                      # sk tiles per score group ([128, 1024] psum)
KT = DIM // 128              # contraction tiles
VW = 130                     # v_sb tile: [0(32) | 1 | 0(31) | v(64) | 1 | pad]

# RoPE layout: within each head's 64 dims -> 64 partitions, quadrant q (32)
# holds pairs 16q..16q+15 as [evens(16) | odds(16)].
_perm = np.empty(64, np.int64)
_freq = np.empty(64, np.int64)
_sgn = np.empty(64, np.float32)
for _p in range(64):
    _q, _j = divmod(_p, 32)
    if _j < 16:
        _i = 16 * _q + _j
        _perm[_p] = 2 * _i
        _sgn[_p] = -1.0
    else:
        _i = 16 * _q + _j - 16
        _perm[_p] = 2 * _i + 1
        _sgn[_p] = 1.0
    _freq[_p] = _i
SHUF = list(range(16, 32)) + list(range(0, 16))

_build_cache = {}
last_exec_time_ns = None
last_trace = None


def _mask_structure(mask):
    """Returns (chunks, patterns): chunks[c] = [(t0, pat_idx|None), ...] over
    groups of GRP sk-tiles; patterns = list of [128, GRP*SQC] float32 0/1."""
    valid = mask[0, 0] == 0.0  # [sq, sk]
    chunks = []
    patterns = []
    pat_keys = {}
    for c in range(NCHUNK):
        glist = []
        for t0 in range(0, NSKT, GRP):
            sub = valid[c * SQC:(c + 1) * SQC, t0 * SKT:(t0 + GRP) * SKT]
            if not sub.any():
                continue
            if sub.all():
                glist.append((t0, None))
                continue
            pat = np.empty((128, GRP * SQC), np.float32)
            for u in range(GRP):
                pat[:, u * SQC:(u + 1) * SQC] = (
                    valid[c * SQC:(c + 1) * SQC,
                          (t0 + u) * SKT:(t0 + u + 1) * SKT].T
                )
            key = pat.tobytes()
            if key not in pat_keys:
                pat_keys[key] = len(patterns)
                patterns.append(pat)
            glist.append((t0, pat_keys[key]))
        chunks.append(glist)
    return chunks, patterns


def _build(chunks, n_pat):
    import concourse.bass as bass  # noqa: F401
    import concourse.mybir as mybir
    from concourse import bacc
    from concourse.tile import TileContext

    F32, BF = mybir.dt.float32, mybir.dt.bfloat16
    MUL = mybir.AluOpType.mult
    ADD = mybir.AluOpType.add
    EXP = mybir.ActivationFunctionType.Exp

    nc = bacc.Bacc()
    xt_e = nc.declare_dram_parameter("xt", [DIM, S], BF, isOutput=False)
    wq_e = nc.declare_dram_parameter("wq", [DIM, HQ], BF, isOutput=False)
    wk_e = nc.declare_dram_parameter("wk", [DIM, HKV], BF, isOutput=False)
    wv_e = nc.declare_dram_parameter("wv", [DIM, HKV], BF, isOutput=False)
    wo_e = nc.declare_dram_parameter("wo", [HQ, DIM], BF, isOutput=False)
    c1_e = nc.declare_dram_parameter("c1", [128, S], BF, isOutput=False)
    c2_e = nc.declare_dram_parameter("c2", [128, S], BF, isOutput=False)
    dm_e = nc.declare_dram_parameter("dmask", [128, n_pat * GRP * SQC], BF,
                                     isOutput=False)
    out_e = nc.declare_dram_parameter("out", [S, DIM], F32, isOutput=True)

    with TileContext(nc) as tc:
        with tc.tile_pool(name="persist", bufs=1) as P:
            q_t = [P.tile([128, S], BF, tag=f"q{j}", name=f"q{j}")
                   for j in range(NPAIR)]
            k_t = P.tile([128, S], BF, tag="kt")
            v_sb = [P.tile([128, NSKT * VW], BF, tag=f"v{g}", name=f"v{g}")
                    for g in range(NKVC)]
            attn = [P.tile([128, S], BF, tag=f"a{j}", name=f"a{j}")
                    for j in range(NPAIR)]
            wo_sb = [P.tile([128, DIM], BF, tag=f"wo{j}", name=f"wo{j}")
                     for j in range(NPAIR)]
            dm_sb = P.tile([128, n_pat * GRP * SQC], BF, tag="dm")

            for j in range(NPAIR):
                nc.sync.dma_start(out=wo_sb[j],
                                  in_=wo_e[128 * j:128 * (j + 1), :])
            nc.sync.dma_start(out=dm_sb, in_=dm_e[:, :])

            # v background: [0(63) | 1 | v | 1 | pad] per sk-tile block
            for g in range(NKVC):
                v3 = v_sb[g].rearrange("p (t w) -> p t w", w=VW)
                nc.vector.memset(v3[:, :, 0:32], 0.0)
                nc.vector.memset(v3[:, :, 32:33], 1.0)
                nc.vector.memset(v3[:, :, 33:64], 0.0)
                nc.vector.memset(v3[:, :, 128:129], 1.0)

            # ---------------- projections ----------------
            with (
                tc.tile_pool(name="xw", bufs=1) as XW,
                tc.tile_pool(name="ropew", bufs=2) as W,
                tc.tile_pool(name="pps", bufs=2, space="PSUM") as PPS,
            ):
                c1_sb = XW.tile([128, S], BF, tag="c1")
                c2_sb = XW.tile([128, S], BF, tag="c2")
                nc.sync.dma_start(out=c1_sb, in_=c1_e[:, :])
                nc.sync.dma_start(out=c2_sb, in_=c2_e[:, :])
                # interleave per-k loads so the first q-proj matmuls can
                # start as soon as (wq[0], x[0]) land
                xt_sb, wq_sb, wk_sb, wv_sb = [], [], [], []
                for k in range(KT):
                    qk_ = XW.tile([128, HQ], BF, tag=f"wq{k}")
                    nc.sync.dma_start(out=qk_,
                                      in_=wq_e[128 * k:128 * (k + 1), :])
                    wq_sb.append(qk_)
                    xk = XW.tile([128, S], BF, tag=f"x{k}")
                    nc.sync.dma_start(out=xk,
                                      in_=xt_e[128 * k:128 * (k + 1), :])
                    xt_sb.append(xk)
                    kk = XW.tile([128, HKV], BF, tag=f"wk{k}")
                    nc.sync.dma_start(out=kk,
                                      in_=wk_e[128 * k:128 * (k + 1), :])
                    wk_sb.append(kk)
                    vk = XW.tile([128, HKV], BF, tag=f"wv{k}")
                    nc.sync.dma_start(out=vk,
                                      in_=wv_e[128 * k:128 * (k + 1), :])
                    wv_sb.append(vk)

                def rope_project(dst, w_tiles, col0):
                    # k-outer: one ldweights feeds 4 chunk matmuls, and the
                    # first matmul only needs (w[0], x[0]) loaded
                    raw = W.tile([128, S], BF, tag="qraw")
                    pss = [PPS.tile([128, SQC], F32, tag=f"pq{c}",
                                    name=f"pq{c}", bufs=1)
                           for c in range(NCHUNK)]
                    for k in range(KT):
                        for c in range(NCHUNK):
                            nc.tensor.matmul(
                                pss[c],
                                w_tiles[k][:, col0:col0 + 128],
                                xt_sb[k][:, SQC * c:SQC * (c + 1)],
                                start=(k == 0), stop=(k == KT - 1),
                            )
                    for c in range(NCHUNK):
                        nc.vector.tensor_copy(raw[:, SQC * c:SQC * (c + 1)],
                                              pss[c])
                    sh = W.tile([128, S], BF, tag="sh")
                    t1 = W.tile([128, S], BF, tag="t1")
                    nc.vector.stream_shuffle(sh, raw, SHUF)
                    nc.vector.tensor_tensor(t1, raw, c1_sb, MUL)
                    nc.vector.tensor_tensor(sh, sh, c2_sb, MUL)
                    nc.vector.tensor_tensor(dst, t1, sh, ADD)

                for j in range(NPAIR):
                    rope_project(q_t[j], wq_sb, 128 * j)
                rope_project(k_t, wk_sb, 0)

                for t in range(NSKT):
                    psv = PPS.tile([128, HKV], F32, tag="pv")
                    for k in range(KT):
                        nc.tensor.matmul(
                            psv,
                            xt_sb[k][:, SKT * t:SKT * (t + 1)],
                            wv_sb[k],
                            start=(k == 0), stop=(k == KT - 1),
                        )
                    for g in range(NKVC):
                        nc.vector.tensor_copy(
                            v_sb[g][:, VW * t + 64:VW * t + 128],
                            psv[:, 64 * g:64 * (g + 1)],
                        )

            # ---------------- attention + wo ----------------
            with (
                tc.tile_pool(name="attw", bufs=2) as W,
                tc.tile_pool(name="scps", bufs=2, space="PSUM") as SCPS,
                tc.tile_pool(name="avps", bufs=2, space="PSUM") as AVPS,
                tc.tile_pool(name="ops", bufs=2, space="PSUM") as OPS,
            ):
                def wo_tile(s):
                    o_sb = W.tile([128, DIM], F32, tag="osb")
                    for n in range(DIM // 512):
                        pso = OPS.tile([128, 512], F32, tag="pso")
                        for j in range(NPAIR):
                            nc.tensor.matmul(
                                pso,
                                attn[j][:, 128 * s:128 * (s + 1)],
                                wo_sb[j][:, 512 * n:512 * (n + 1)],
                                start=(j == 0), stop=(j == NPAIR - 1),
                            )
                        nc.vector.tensor_copy(o_sb[:, 512 * n:512 * (n + 1)],
                                              pso)
                    nc.gpsimd.dma_start(out=out_e[128 * s:128 * (s + 1), :],
                                        in_=o_sb)

                for c in range(NCHUNK):
                    glist = chunks[c]
                    for j in range(NPAIR):
                        # pair j = (q-head j -> kv 0, q-head j+4 -> kv 1)
                        for half in (0, 1):
                            rows = slice(64 * half, 64 * (half + 1))
                            av = AVPS.tile([128, SQC], F32, tag="av",
                                           name="av")
                            for gi, (t0, patk) in enumerate(glist):
                                first = gi == 0
                                last = gi == len(glist) - 1
                                sc = SCPS.tile([128, GRP * SQC], F32,
                                               tag="sc", name="sc")
                                for u in range(GRP):
                                    t = t0 + u
                                    nc.tensor.matmul(
                                        sc[:, SQC * u:SQC * (u + 1)],
                                        k_t[rows, SKT * t:SKT * (t + 1)],
                                        q_t[j][rows, SQC * c:SQC * (c + 1)],
                                        start=True, stop=True,
                                    )
                                p = W.tile([128, GRP * SQC], BF,
                                           tag=f"p{half}", name="p")
                                nc.scalar.activation(p, sc, EXP, scale=0.125)
                                if patk is not None:
                                    dslice = dm_sb[:, GRP * SQC * patk:
                                                   GRP * SQC * (patk + 1)]
                                    nc.vector.tensor_tensor(p, p, dslice, MUL)
                                for u in range(GRP):
                                    t = t0 + u
                                    if half == 0:
                                        vv = v_sb[0][:, VW * t + 64:
                                                     VW * t + 129]
                                        o = av[0:65, :]
                                    else:
                                        vv = v_sb[1][:, VW * t:VW * t + 128]
                                        o = av[0:128, :]
                                    nc.tensor.matmul(
                                        o, vv, p[:, SQC * u:SQC * (u + 1)],
                                        start=(first and u == 0),
                                        stop=(last and u == GRP - 1),
                                    )
                            rec = W.tile([1, SQC], F32, tag=f"rec{half}",
                                         name="rec")
                            rb = W.tile([128, SQC], F32, tag=f"rb{half}",
                                        name="rb")
                            if half == 0:
                                nc.vector.reciprocal(rec[0:1, :],
                                                     av[64:65, :])
                                nc.gpsimd.partition_broadcast(rb, rec[0:1, :])
                                nc.vector.tensor_tensor(
                                    attn[j][0:64, SQC * c:SQC * (c + 1)],
                                    av[0:64, :], rb[0:64, :], MUL)
                            else:
                                nc.vector.reciprocal(rec[0:1, :],
                                                     av[32:33, :])
                                nc.gpsimd.partition_broadcast(rb, rec[0:1, :])
                                nc.vector.tensor_tensor(
                                    attn[j][64:128, SQC * c:SQC * (c + 1)],
                                    av[64:128, :], rb[64:128, :], MUL)
                        if c > 0:
                            wo_tile(4 * (c - 1) + j)
                for j in range(NPAIR):
                    wo_tile(4 * (NCHUNK - 1) + j)

    nc.finalize()
    return nc


def kernel(**inputs):
    global last_exec_time_ns, last_trace
    from concourse.bass_utils import run_bass_kernel_spmd

    x = np.asarray(inputs["x"], np.float32)
    freqs_cos = np.asarray(inputs["freqs_cos"], np.float32)
    freqs_sin = np.asarray(inputs["freqs_sin"], np.float32)
    mask = np.asarray(inputs["mask"], np.float32)
    wq = np.asarray(inputs["wq"], np.float32)
    wk = np.asarray(inputs["wk"], np.float32)
    wv = np.asarray(inputs["wv"], np.float32)
    wo = np.asarray(inputs["wo"], np.float32)

    chunks, patterns = _mask_structure(mask)
    n_pat = max(len(patterns), 1)
    if patterns:
        dmask = np.concatenate(patterns, axis=1).astype(BF16)
    else:
        dmask = np.ones((128, GRP * SQC), np.float32).astype(BF16)

    key = tuple(tuple(g) for g in chunks)
    if key not in _build_cache:
        _build_cache[key] = _build(chunks, n_pat)
    nc = _build_cache[key]

    # trig tiles in pair layout (same for both heads of a pair)
    fi2 = np.tile(_freq, 2)
    sg2 = np.tile(_sgn, 2)
    c1 = freqs_cos.T[fi2].astype(BF16)                      # [128, S]
    c2 = (freqs_sin.T[fi2] * sg2[:, None]).astype(BF16)     # [128, S]

    # pair j holds (q-head j, q-head j+4) so lo half uses kv 0, hi half kv 1
    pair_order = [0, 4, 1, 5, 2, 6, 3, 7]
    q_cols = np.concatenate([64 * pair_order[i] + _perm
                             for i in range(H // TP)])
    o_rows = np.concatenate([np.arange(64 * pair_order[i],
                                       64 * pair_order[i] + 64)
                             for i in range(H // TP)])
    kv_perm = np.concatenate([64 * h + _perm for h in range(KV // TP)])

    in_maps = []
    for d in range(DP):
        xt = np.ascontiguousarray(x[d].T).astype(BF16)
        for t in range(TP):
            wq_s = np.ascontiguousarray(
                wq[:, HQ * t:HQ * (t + 1)][:, q_cols]).astype(BF16)
            wk_s = np.ascontiguousarray(
                wk[:, HKV * t:HKV * (t + 1)][:, kv_perm]).astype(BF16)
            wv_s = np.ascontiguousarray(
                wv[:, HKV * t:HKV * (t + 1)]).astype(BF16)
            wo_s = np.ascontiguousarray(
                wo[HQ * t:HQ * (t + 1), :][o_rows]).astype(BF16)
            in_maps.append({
                "xt": xt, "wq": wq_s, "wk": wk_s, "wv": wv_s, "wo": wo_s,
                "c1": c1, "c2": c2, "dmask": dmask,
            })

    trace = bool(os.environ.get("BASS_KERNEL_TRACE"))
    res = run_bass_kernel_spmd(nc, in_maps, core_ids=list(range(NCORES)),
                               trace=trace)
    last_exec_time_ns = res.exec_time_ns
    last_trace = res
    out = np.empty((B, S, DIM), np.float32)
    for d in range(DP):
        acc = res.results[d * TP]["out"].astype(np.float32)
        for t in range(1, TP):
            acc = acc + res.results[d * TP + t]["out"]
        out[d] = acc
    return out
